# revision 5
# baseline (speedup 1.0000x reference)
"""Chamfer distance via banded exact nearest-neighbor for Trainium2.

Host side: for each batch and each direction (q=set1 vs c=set2 and the
swap), sort candidates by x and compute for every query a provable upper
bound u(q) on its NN distance (exact distance to the best of ~128 real
candidate points found via x-window + Morton-window).  The true NN then
provably lies in the sorted-candidate index band {m : |x_m - x_q| <= u(q)}.
The device only computes distance tiles over those bands.

Queries are processed in chunks of 128 (sorted order); a chunk's band is
the union of its members' bands, excluding "outlier" queries with wide
bands, which are re-processed in dedicated gathered outlier chunks.  Every
computed value is a distance to a real candidate, so a host-side min-merge
of all contributions per query is exact (up to fp32r matmul rounding).

Device per segment (band split into <=1024-wide pieces):
  - K=5 augmented matmul [5,128]x[5,W] -> PSUM fp32 (fp32r, 1 cycle/row)
  - route A (DVE): tensor_reduce min directly from PSUM -> dist col
  - route B (ACT+DVE): ACT Relu-cast PSUM->SBUF f16, then DVE
    tensor_scalar min-accum in 4x mode
  routes are balanced greedily between the ACT and DVE engines.
No staging of the full distance matrix, no transpose tail.
"""

import sys

sys.path.insert(0, "/opt/trn_rl_repo")

import numpy as np

import concourse.bass as bass  # noqa: F401
import concourse.mybir as mybir
import concourse.tile as tile
from concourse import bacc
from concourse.bass_utils import run_bass_kernel_spmd

B, N, M = 8, 8192, 8192
P = 128
NCH = N // P          # 64 normal chunks per side
GRAN = 128            # band width granularity (<=512 widths)
WSEG = 1024           # max segment width (psum tile = 2 banks)
T_OUT = 320           # per-query band width above which query is an outlier
KX = 192              # x-window candidates for u(q)
KM = 192              # morton-window candidates for u(q)

F32 = mybir.dt.float32
F32R = mybir.dt.float32r
F16 = mybir.dt.float16


# ----------------------------------------------------------------- host plan

def _morton(p):
    pm = (p - p.min(0)) / (p.max(0) - p.min(0) + 1e-12)
    g = np.minimum((pm * 1024).astype(np.int64), 1023)

    def spread(x):
        x = (x | (x << 16)) & 0x030000FF
        x = (x | (x << 8)) & 0x0300F00F
        x = (x | (x << 4)) & 0x030C30C3
        x = (x | (x << 2)) & 0x09249249
        return x

    return (spread(g[:, 0]) << 2) | (spread(g[:, 1]) << 1) | spread(g[:, 2])


def _bands(q, c):
    """q,c float64 [n,3]. Returns qi, ci, blo, bhi (bands of sorted queries
    in sorted-candidate index space, provably containing each query's NN)."""
    n, m = len(q), len(c)
    qi = np.argsort(q[:, 0], kind="stable")
    ci = np.argsort(c[:, 0], kind="stable")
    qs, cs = q[qi], c[ci]
    xq, xc = qs[:, 0], cs[:, 0]
    pos = np.searchsorted(xc, xq)
    lo = np.clip(pos - KX // 2, 0, m - KX)
    idx = lo[:, None] + np.arange(KX)[None, :]
    u2 = ((qs[:, None, :] - cs[idx]) ** 2).sum(-1).min(1)
    mq, mc = _morton(qs), _morton(cs)
    mci = np.argsort(mc)
    posm = np.searchsorted(mc[mci], mq)
    lom = np.clip(posm - KM // 2, 0, m - KM)
    idxm = mci[lom[:, None] + np.arange(KM)[None, :]]
    u2 = np.minimum(u2, ((qs[:, None, :] - cs[idxm]) ** 2).sum(-1).min(1))
    u = np.sqrt(u2) * (1 + 1e-9) + 1e-12
    blo = np.searchsorted(xc, xq - u, "left")
    bhi = np.searchsorted(xc, xq + u, "right")
    return qi, ci, blo, bhi


def _pad(w):
    w = int(w)
    if w <= 512:
        return max(256, -(-w // GRAN) * GRAN)
    return -(-w // 256) * 256


def _plan_side(batches):
    """batches: list of (qi, ci, blo, bhi) per batch.  Returns a shared chunk
    plan + per-batch gather/merge info.

    chunk list entries: (kind, index, W); per batch: per chunk candidate
    slice start l_b, and for outlier chunks the lane->sorted-query mapping.
    """
    nb = len(batches)
    # normal chunks
    W_norm = np.zeros(NCH, np.int64)
    l_norm = np.zeros((nb, NCH), np.int64)
    for b, (qi, ci, blo, bhi) in enumerate(batches):
        w = bhi - blo
        inl = w <= T_OUT
        for ch in range(NCH):
            s = slice(ch * P, (ch + 1) * P)
            msk = inl[s]
            if msk.any():
                l, h = blo[s][msk].min(), bhi[s][msk].max()
            else:
                l, h = blo[s].min(), blo[s].min() + GRAN
            l_norm[b, ch] = l
            W_norm[ch] = max(W_norm[ch], _pad(h - l))
    W_norm = np.minimum(W_norm, M)
    l_norm = np.minimum(l_norm, M - W_norm[None, :])
    np.clip(l_norm, 0, None, out=l_norm)

    # outlier groups per batch
    out_groups = []  # per batch: list of (sorted_positions[<=P], l, need)
    for b, (qi, ci, blo, bhi) in enumerate(batches):
        w = bhi - blo
        oq = np.where(w > T_OUT)[0]
        oq = oq[np.argsort(blo[oq], kind="stable")]
        groups = []
        for i in range(0, len(oq), P):
            grp = oq[i:i + P]
            l, h = blo[grp].min(), bhi[grp].max()
            groups.append((grp, int(l), int(h - l)))
        out_groups.append(groups)
    K = max(len(g) for g in out_groups)
    W_out = np.zeros(K, np.int64)
    l_out = np.zeros((nb, K), np.int64)
    lanes_out = np.zeros((nb, K, P), np.int64)  # sorted-query positions
    for b in range(nb):
        for g in range(K):
            if g < len(out_groups[b]):
                grp, l, need = out_groups[b][g]
            else:
                grp, l, need = np.array([0]), 0, GRAN
            lane = np.empty(P, np.int64)
            lane[:len(grp)] = grp
            lane[len(grp):] = grp[-1]
            lanes_out[b, g] = lane
            l_out[b, g] = l
            W_out[g] = max(W_out[g], _pad(need))
    W_out = np.minimum(W_out, M)
    l_out = np.minimum(l_out, M - W_out[None, :])
    np.clip(l_out, 0, None, out=l_out)

    # segment list (shared across batches)
    segs = []  # (qoff_rel, coff_rel, w)  qoff_rel in side Q tensor
    coff = 0
    for ch in range(NCH):
        Wc = int(W_norm[ch])
        for j in range(0, Wc, WSEG):
            segs.append((ch * P, coff + j, min(WSEG, Wc - j)))
        coff += Wc
    for g in range(K):
        Wc = int(W_out[g])
        for j in range(0, Wc, WSEG):
            segs.append((N + g * P, coff + j, min(WSEG, Wc - j)))
        coff += Wc

    return dict(K=K, QTOT=N + K * P, CTOT=coff, segs=segs,
                W_norm=W_norm, l_norm=l_norm,
                W_out=W_out, l_out=l_out, lanes_out=lanes_out)


def _aug_q(p):
    """p [n,3] fp32 -> [5,n]: x,y,z,|p|^2,1"""
    a = np.empty((5, len(p)), np.float32)
    a[0:3] = p.T
    a[3] = (p * p).sum(1)
    a[4] = 1.0
    return a


def _aug_c(p):
    """p [m,3] fp32 -> [5,m]: -2x,-2y,-2z,1,|p|^2"""
    a = np.empty((5, len(p)), np.float32)
    a[0:3] = -2.0 * p.T
    a[3] = 1.0
    a[4] = (p * p).sum(1)
    return a


def _gather_side(plan, qi, ci, q32, c32):
    """Build per-batch Q [5,QTOT] and C [5,CTOT] fp32 for one side/batch."""
    b = None  # noqa
    qs = q32[qi]
    cs = c32[ci]
    augq = _aug_q(qs)
    augc = _aug_c(cs)
    Q = np.empty((5, plan["QTOT"]), np.float32)
    Q[:, :N] = augq
    C = np.empty((5, plan["CTOT"]), np.float32)
    return Q, C, augq, augc, cs


# ------------------------------------------------------------- device build

_cache = {}


def build(segs_a, segs_b, qtot_a, qtot_b, ctot_a, ctot_b):
    key = (tuple(segs_a), tuple(segs_b), qtot_a, qtot_b, ctot_a, ctot_b)
    if key in _cache:
        return _cache[key]
    CMAX = max(ctot_a, ctot_b)
    S = len(segs_a) + len(segs_b)

    act_t = 0.0
    dve_t = 0.0
    routes = []
    for (_, _, w) in list(segs_a) + list(segs_b):
        c_dir = w * 1.042 + 190.0
        c_act = w * 0.833 + (160.0 if w <= 512 else 245.0)
        c_tsd = w * 0.26 + 130.0
        m_dir = max(act_t, dve_t + c_dir)
        m_act = max(act_t + c_act, dve_t + c_tsd)
        if m_act <= m_dir:
            routes.append(1)
            act_t += c_act
            dve_t += c_tsd
        else:
            routes.append(0)
            dve_t += c_dir

    nc = bacc.Bacc()
    qA = nc.declare_dram_parameter("qA", [5, qtot_a], F32R, isOutput=False)
    cA = nc.declare_dram_parameter("cA", [5, ctot_a], F32R, isOutput=False)
    qB = nc.declare_dram_parameter("qB", [5, qtot_b], F32R, isOutput=False)
    cB = nc.declare_dram_parameter("cB", [5, ctot_b], F32R, isOutput=False)
    dout = nc.declare_dram_parameter("dout", [P, S], F32, isOutput=True)

    with tile.TileContext(nc) as tc:
        with tc.tile_pool(name="const", bufs=1) as const, \
             tc.tile_pool(name="st", bufs=5) as stp, \
             tc.tile_pool(name="psum1", bufs=2, space="PSUM") as psum1, \
             tc.tile_pool(name="psum2", bufs=3, space="PSUM") as psum2:

            QMAX = max(qtot_a, qtot_b)
            q_sb = const.tile([5, QMAX], F32R)
            c_sb = const.tile([5, CMAX], F32R)
            dist_sb = const.tile([P, S], F32)

            s = 0
            qi_ = 0
            for phase, (segs, qT, qtot, cT, ctot) in enumerate(
                    ((segs_a, qA, qtot_a, cA, ctot_a),
                     (segs_b, qB, qtot_b, cB, ctot_b))):
                nc.sync.dma_start(out=q_sb[:, :qtot], in_=qT[:])
                # candidate DMA in graded pieces so early chunks start sooner
                cuts = [0, ctot // 16, ctot // 8, ctot // 4, ctot // 2, ctot]
                for a, b2 in zip(cuts[:-1], cuts[1:]):
                    if a < b2:
                        nc.sync.dma_start(out=c_sb[:, a:b2], in_=cT[:, a:b2])

                def emit_ts(p):
                    ap_, w_, s_ = p
                    nc.vector.tensor_scalar(
                        out=ap_, in0=ap_,
                        scalar1=0.0, scalar2=None,
                        op0=mybir.AluOpType.max, op1=mybir.AluOpType.min,
                        accum_out=dist_sb[:, s_:s_ + 1],
                    )

                pending = []   # delayed route-1 TS ops (DVE keeps busy
                # while ACT casts); route-1 w<=512 segs are PAIRED into one
                # 2-bank psum tile with a single ACT cast (the slot gap casts
                # stale psum data that no TS ever reads)
                pbuf = []

                def flush_pair(force=False):
                    if not pbuf or (len(pbuf) < 2 and not force):
                        return
                    grp = pbuf[:2]
                    del pbuf[:len(grp)]
                    ps = psum2.tile([P, WSEG], F32, tag="ps2")
                    for k, (qoff_, coff_, w_, s_) in enumerate(grp):
                        nc.tensor.matmul(
                            out=ps[:, k * 512:k * 512 + w_],
                            lhsT=q_sb[:, qoff_:qoff_ + P],
                            rhs=c_sb[:, coff_:coff_ + w_],
                            start=True, stop=True,
                        )
                    span = (len(grp) - 1) * 512 + grp[-1][2]
                    st = stp.tile([P, WSEG], F16, tag="st")
                    nc.scalar.activation(
                        out=st[:, :span], in_=ps[:, :span],
                        func=mybir.ActivationFunctionType.Relu,
                    )
                    while pending:
                        emit_ts(pending.pop(0))
                    for k, (qoff_, coff_, w_, s_) in enumerate(grp):
                        pending.append((st[:, k * 512:k * 512 + w_], w_, s_))

                for (qoff, coff, w) in segs:
                    route = routes[s]
                    if route == 1 and w <= 512:
                        pbuf.append((qoff, coff, w, s))
                        flush_pair()
                        s += 1
                        continue
                    if w <= 512:
                        ps = psum1.tile([P, 512], F32, tag="ps1")
                    else:
                        ps = psum2.tile([P, WSEG], F32, tag="ps2")
                    for j in range(0, w, 512):
                        j1 = min(j + 512, w)
                        nc.tensor.matmul(
                            out=ps[:, j:j1],
                            lhsT=q_sb[:, qoff:qoff + P],
                            rhs=c_sb[:, coff + j:coff + j1],
                            start=True, stop=True,
                        )
                    if route == 0:
                        nc.vector.tensor_reduce(
                            out=dist_sb[:, s:s + 1], in_=ps[:, :w],
                            axis=mybir.AxisListType.X, op=mybir.AluOpType.min,
                        )
                        if pending:
                            emit_ts(pending.pop(0))
                    else:
                        st = stp.tile([P, WSEG], F16, tag="st")
                        nc.scalar.activation(
                            out=st[:, :w], in_=ps[:, :w],
                            func=mybir.ActivationFunctionType.Relu,
                        )
                        if pending:
                            emit_ts(pending.pop(0))
                        pending.append((st[:, :w], w, s))
                    s += 1
                flush_pair(force=True)
                while pending:
                    emit_ts(pending.pop(0))

            # final relu (covers route-0 segments; min commutes with relu)
            nc.vector.tensor_scalar(
                out=dist_sb[:], in0=dist_sb[:], scalar1=0.0, scalar2=None,
                op0=mybir.AluOpType.max,
            )
            nc.sync.dma_start(out=dout[:], in_=dist_sb[:])

    nc.finalize()
    _cache[key] = nc
    return nc


# ------------------------------------------------------------------- driver

def _prepare(input1, input2):
    p1 = np.ascontiguousarray(np.asarray(input1, np.float32))
    p2 = np.ascontiguousarray(np.asarray(input2, np.float32))
    sides = []  # side 0: q=p1,c=p2 (dist1); side 1: q=p2,c=p1 (dist2)
    for (qa, ca) in ((p1, p2), (p2, p1)):
        binfo = [_bands(qa[b].astype(np.float64), ca[b].astype(np.float64))
                 for b in range(B)]
        plan = _plan_side(binfo)
        sides.append((binfo, plan, qa, ca))
    return sides


def run(input1, input2, trace=False):
    sides = _prepare(input1, input2)
    (binfo_a, plan_a, q_a, c_a) = sides[0]
    (binfo_b, plan_b, q_b, c_b) = sides[1]

    nc = build(plan_a["segs"], plan_b["segs"],
               plan_a["QTOT"], plan_b["QTOT"],
               plan_a["CTOT"], plan_b["CTOT"])
    global LAST_NC
    LAST_NC = nc

    in_maps = []
    for b in range(B):
        im = {}
        for name_q, name_c, (binfo, plan, qq, cc) in (
                ("qA", "cA", (binfo_a, plan_a, q_a, c_a)),
                ("qB", "cB", (binfo_b, plan_b, q_b, c_b))):
            qi, ci, blo, bhi = binfo[b]
            qs = qq[b][qi]
            cs = cc[b][ci]
            augq = _aug_q(qs)
            augc = _aug_c(cs)
            Q = np.empty((5, plan["QTOT"]), np.float32)
            Q[:, :N] = augq
            for g in range(plan["K"]):
                Q[:, N + g * P:N + (g + 1) * P] = augq[:, plan["lanes_out"][b, g]]
            C = np.empty((5, plan["CTOT"]), np.float32)
            off = 0
            for ch in range(NCH):
                W = int(plan["W_norm"][ch])
                l = int(plan["l_norm"][b, ch])
                C[:, off:off + W] = augc[:, l:l + W]
                off += W
            for g in range(plan["K"]):
                W = int(plan["W_out"][g])
                l = int(plan["l_out"][b, g])
                C[:, off:off + W] = augc[:, l:l + W]
                off += W
            im[name_q] = Q
            im[name_c] = C
        in_maps.append(im)

    res = run_bass_kernel_spmd(nc, in_maps, list(range(B)), trace=trace)

    # host merge
    sa = len(plan_a["segs"])
    dist1 = np.empty((B, N), np.float32)
    dist2 = np.empty((B, N), np.float32)
    for b in range(B):
        out = res.results[b]["dout"]  # [P, S]
        for side, (binfo, plan, dst) in enumerate(
                ((binfo_a, plan_a, dist1), (binfo_b, plan_b, dist2))):
            qi = binfo[b][0]
            segs = plan["segs"]
            cols = out[:, :sa] if side == 0 else out[:, sa:]
            acc = np.full(N, np.inf, np.float32)
            ids = np.empty((len(segs), P), np.int64)
            for s, (qoff, _, _) in enumerate(segs):
                if qoff < N:
                    ids[s] = qi[qoff:qoff + P]
                else:
                    g = (qoff - N) // P
                    ids[s] = qi[plan["lanes_out"][b, g]]
            np.minimum.at(acc, ids.reshape(-1), cols.T.reshape(-1))
            dst[b] = acc
    return (dist1, dist2), res


def kernel(input1, input2):
    (dist1, dist2), _ = run(input1, input2)
    return (dist1, dist2)


# revision 6
# speedup vs baseline: 1.0009x; 1.0009x over previous
"""Chamfer distance via banded exact nearest-neighbor for Trainium2.

Host side: for each batch and each direction (q=set1 vs c=set2 and the
swap), sort candidates by x and compute for every query a provable upper
bound u(q) on its NN distance (exact distance to the best of ~128 real
candidate points found via x-window + Morton-window).  The true NN then
provably lies in the sorted-candidate index band {m : |x_m - x_q| <= u(q)}.
The device only computes distance tiles over those bands.

Queries are processed in chunks of 128 (sorted order); a chunk's band is
the union of its members' bands, excluding "outlier" queries with wide
bands, which are re-processed in dedicated gathered outlier chunks.  Every
computed value is a distance to a real candidate, so a host-side min-merge
of all contributions per query is exact (up to fp32r matmul rounding).

Device per segment (band split into <=1024-wide pieces):
  - K=5 augmented matmul [5,128]x[5,W] -> PSUM fp32 (fp32r, 1 cycle/row)
  - route A (DVE): tensor_reduce min directly from PSUM -> dist col
  - route B (ACT+DVE): ACT Relu-cast PSUM->SBUF f16, then DVE
    tensor_scalar min-accum in 4x mode
  routes are balanced greedily between the ACT and DVE engines.
No staging of the full distance matrix, no transpose tail.
"""

import sys

sys.path.insert(0, "/opt/trn_rl_repo")

import numpy as np

import concourse.bass as bass  # noqa: F401
import concourse.mybir as mybir
import concourse.tile as tile
from concourse import bacc
from concourse.bass_utils import run_bass_kernel_spmd

B, N, M = 8, 8192, 8192
P = 128
NCH = N // P          # 64 normal chunks per side
GRAN = 128            # band width granularity (<=512 widths)
WSEG = 1024           # max segment width (psum tile = 2 banks)
T_OUT = 320           # per-query band width above which query is an outlier
KX = 192              # x-window candidates for u(q)
KM = 192              # morton-window candidates for u(q)

F32 = mybir.dt.float32
F32R = mybir.dt.float32r
F16 = mybir.dt.float16


# ----------------------------------------------------------------- host plan

def _morton(p):
    pm = (p - p.min(0)) / (p.max(0) - p.min(0) + 1e-12)
    g = np.minimum((pm * 1024).astype(np.int64), 1023)

    def spread(x):
        x = (x | (x << 16)) & 0x030000FF
        x = (x | (x << 8)) & 0x0300F00F
        x = (x | (x << 4)) & 0x030C30C3
        x = (x | (x << 2)) & 0x09249249
        return x

    return (spread(g[:, 0]) << 2) | (spread(g[:, 1]) << 1) | spread(g[:, 2])


def _bands(q, c):
    """q,c float64 [n,3]. Returns qi, ci, blo, bhi (bands of sorted queries
    in sorted-candidate index space, provably containing each query's NN)."""
    n, m = len(q), len(c)
    qi = np.argsort(q[:, 0], kind="stable")
    ci = np.argsort(c[:, 0], kind="stable")
    qs, cs = q[qi], c[ci]
    xq, xc = qs[:, 0], cs[:, 0]
    pos = np.searchsorted(xc, xq)
    lo = np.clip(pos - KX // 2, 0, m - KX)
    idx = lo[:, None] + np.arange(KX)[None, :]
    u2 = ((qs[:, None, :] - cs[idx]) ** 2).sum(-1).min(1)
    mq, mc = _morton(qs), _morton(cs)
    mci = np.argsort(mc)
    posm = np.searchsorted(mc[mci], mq)
    lom = np.clip(posm - KM // 2, 0, m - KM)
    idxm = mci[lom[:, None] + np.arange(KM)[None, :]]
    u2 = np.minimum(u2, ((qs[:, None, :] - cs[idxm]) ** 2).sum(-1).min(1))
    u = np.sqrt(u2) * (1 + 1e-9) + 1e-12
    blo = np.searchsorted(xc, xq - u, "left")
    bhi = np.searchsorted(xc, xq + u, "right")
    return qi, ci, blo, bhi


def _pad(w):
    w = int(w)
    if w <= 512:
        return max(256, -(-w // GRAN) * GRAN)
    return -(-w // 256) * 256


def _plan_side(batches):
    """batches: list of (qi, ci, blo, bhi) per batch.  Returns a shared chunk
    plan + per-batch gather/merge info.

    chunk list entries: (kind, index, W); per batch: per chunk candidate
    slice start l_b, and for outlier chunks the lane->sorted-query mapping.
    """
    nb = len(batches)
    # normal chunks
    W_norm = np.zeros(NCH, np.int64)
    l_norm = np.zeros((nb, NCH), np.int64)
    for b, (qi, ci, blo, bhi) in enumerate(batches):
        w = bhi - blo
        inl = w <= T_OUT
        for ch in range(NCH):
            s = slice(ch * P, (ch + 1) * P)
            msk = inl[s]
            if msk.any():
                l, h = blo[s][msk].min(), bhi[s][msk].max()
            else:
                l, h = blo[s].min(), blo[s].min() + GRAN
            l_norm[b, ch] = l
            W_norm[ch] = max(W_norm[ch], _pad(h - l))
    W_norm = np.minimum(W_norm, M)
    l_norm = np.minimum(l_norm, M - W_norm[None, :])
    np.clip(l_norm, 0, None, out=l_norm)

    # outlier groups per batch
    out_groups = []  # per batch: list of (sorted_positions[<=P], l, need)
    for b, (qi, ci, blo, bhi) in enumerate(batches):
        w = bhi - blo
        oq = np.where(w > T_OUT)[0]
        oq = oq[np.argsort(blo[oq], kind="stable")]
        groups = []
        for i in range(0, len(oq), P):
            grp = oq[i:i + P]
            l, h = blo[grp].min(), bhi[grp].max()
            groups.append((grp, int(l), int(h - l)))
        out_groups.append(groups)
    K = max(len(g) for g in out_groups)
    W_out = np.zeros(K, np.int64)
    l_out = np.zeros((nb, K), np.int64)
    lanes_out = np.zeros((nb, K, P), np.int64)  # sorted-query positions
    for b in range(nb):
        for g in range(K):
            if g < len(out_groups[b]):
                grp, l, need = out_groups[b][g]
            else:
                grp, l, need = np.array([0]), 0, GRAN
            lane = np.empty(P, np.int64)
            lane[:len(grp)] = grp
            lane[len(grp):] = grp[-1]
            lanes_out[b, g] = lane
            l_out[b, g] = l
            W_out[g] = max(W_out[g], _pad(need))
    W_out = np.minimum(W_out, M)
    l_out = np.minimum(l_out, M - W_out[None, :])
    np.clip(l_out, 0, None, out=l_out)

    # segment list (shared across batches)
    segs = []  # (qoff_rel, coff_rel, w)  qoff_rel in side Q tensor
    coff = 0
    for ch in range(NCH):
        Wc = int(W_norm[ch])
        for j in range(0, Wc, WSEG):
            segs.append((ch * P, coff + j, min(WSEG, Wc - j)))
        coff += Wc
    for g in range(K):
        Wc = int(W_out[g])
        for j in range(0, Wc, WSEG):
            segs.append((N + g * P, coff + j, min(WSEG, Wc - j)))
        coff += Wc

    return dict(K=K, QTOT=N + K * P, CTOT=coff, segs=segs,
                W_norm=W_norm, l_norm=l_norm,
                W_out=W_out, l_out=l_out, lanes_out=lanes_out)


def _aug_q(p):
    """p [n,3] fp32 -> [5,n]: x,y,z,|p|^2,1"""
    a = np.empty((5, len(p)), np.float32)
    a[0:3] = p.T
    a[3] = (p * p).sum(1)
    a[4] = 1.0
    return a


def _aug_c(p):
    """p [m,3] fp32 -> [5,m]: -2x,-2y,-2z,1,|p|^2"""
    a = np.empty((5, len(p)), np.float32)
    a[0:3] = -2.0 * p.T
    a[3] = 1.0
    a[4] = (p * p).sum(1)
    return a


def _gather_side(plan, qi, ci, q32, c32):
    """Build per-batch Q [5,QTOT] and C [5,CTOT] fp32 for one side/batch."""
    b = None  # noqa
    qs = q32[qi]
    cs = c32[ci]
    augq = _aug_q(qs)
    augc = _aug_c(cs)
    Q = np.empty((5, plan["QTOT"]), np.float32)
    Q[:, :N] = augq
    C = np.empty((5, plan["CTOT"]), np.float32)
    return Q, C, augq, augc, cs


# ------------------------------------------------------------- device build

_cache = {}


def build(segs_a, segs_b, qtot_a, qtot_b, ctot_a, ctot_b):
    key = (tuple(segs_a), tuple(segs_b), qtot_a, qtot_b, ctot_a, ctot_b)
    if key in _cache:
        return _cache[key]
    CMAX = max(ctot_a, ctot_b)
    S = len(segs_a) + len(segs_b)

    act_t = 0.0
    dve_t = 0.0
    routes = []
    for (_, _, w) in list(segs_a) + list(segs_b):
        c_dir = w * 1.042 + 190.0
        c_act = w * 0.833 + (160.0 if w <= 512 else 245.0)
        c_tsd = w * 0.26 + 130.0
        m_dir = max(act_t, dve_t + c_dir)
        m_act = max(act_t + c_act, dve_t + c_tsd)
        if m_act <= m_dir:
            routes.append(1)
            act_t += c_act
            dve_t += c_tsd
        else:
            routes.append(0)
            dve_t += c_dir

    nc = bacc.Bacc()
    qA = nc.declare_dram_parameter("qA", [5, qtot_a], F32R, isOutput=False)
    cA = nc.declare_dram_parameter("cA", [5, ctot_a], F32R, isOutput=False)
    qB = nc.declare_dram_parameter("qB", [5, qtot_b], F32R, isOutput=False)
    cB = nc.declare_dram_parameter("cB", [5, ctot_b], F32R, isOutput=False)
    dout = nc.declare_dram_parameter("dout", [P, S], F32, isOutput=True)

    with tile.TileContext(nc) as tc:
        with tc.tile_pool(name="const", bufs=1) as const, \
             tc.tile_pool(name="st", bufs=4) as stp, \
             tc.tile_pool(name="psum1", bufs=2, space="PSUM") as psum1, \
             tc.tile_pool(name="psum2", bufs=3, space="PSUM") as psum2:

            QMAX = max(qtot_a, qtot_b)
            q_sb = const.tile([5, QMAX], F32R)
            c_sb = const.tile([5, CMAX], F32R)
            dist_sb = const.tile([P, S], F32)

            s = 0
            qi_ = 0
            for phase, (segs, qT, qtot, cT, ctot) in enumerate(
                    ((segs_a, qA, qtot_a, cA, ctot_a),
                     (segs_b, qB, qtot_b, cB, ctot_b))):
                nc.sync.dma_start(out=q_sb[:, :qtot], in_=qT[:])
                # candidate DMA in graded pieces so early chunks start sooner
                cuts = [0, ctot // 16, ctot // 8, ctot // 4, ctot // 2, ctot]
                for a, b2 in zip(cuts[:-1], cuts[1:]):
                    if a < b2:
                        nc.sync.dma_start(out=c_sb[:, a:b2], in_=cT[:, a:b2])

                def emit_ts(p):
                    ap_, w_, s_ = p
                    nc.vector.tensor_scalar(
                        out=ap_, in0=ap_,
                        scalar1=0.0, scalar2=None,
                        op0=mybir.AluOpType.max, op1=mybir.AluOpType.min,
                        accum_out=dist_sb[:, s_:s_ + 1],
                    )

                pending = []   # delayed route-1 TS ops (DVE keeps busy
                # while ACT casts); route-1 w<=512 segs are PAIRED into one
                # 2-bank psum tile with a single ACT cast (the slot gap casts
                # stale psum data that no TS ever reads)
                pbuf = []

                def flush_pair(force=False):
                    if not pbuf or (len(pbuf) < 2 and not force):
                        return
                    grp = pbuf[:2]
                    del pbuf[:len(grp)]
                    ps = psum2.tile([P, WSEG], F32, tag="ps2")
                    for k, (qoff_, coff_, w_, s_) in enumerate(grp):
                        nc.tensor.matmul(
                            out=ps[:, k * 512:k * 512 + w_],
                            lhsT=q_sb[:, qoff_:qoff_ + P],
                            rhs=c_sb[:, coff_:coff_ + w_],
                            start=True, stop=True,
                        )
                    span = (len(grp) - 1) * 512 + grp[-1][2]
                    st = stp.tile([P, WSEG], F16, tag="st")
                    nc.scalar.activation(
                        out=st[:, :span], in_=ps[:, :span],
                        func=mybir.ActivationFunctionType.Relu,
                    )
                    while pending:
                        emit_ts(pending.pop(0))
                    for k, (qoff_, coff_, w_, s_) in enumerate(grp):
                        pending.append((st[:, k * 512:k * 512 + w_], w_, s_))

                for (qoff, coff, w) in segs:
                    route = routes[s]
                    if route == 1 and w <= 512:
                        pbuf.append((qoff, coff, w, s))
                        flush_pair()
                        s += 1
                        continue
                    if w <= 512:
                        ps = psum1.tile([P, 512], F32, tag="ps1")
                    else:
                        ps = psum2.tile([P, WSEG], F32, tag="ps2")
                    for j in range(0, w, 512):
                        j1 = min(j + 512, w)
                        nc.tensor.matmul(
                            out=ps[:, j:j1],
                            lhsT=q_sb[:, qoff:qoff + P],
                            rhs=c_sb[:, coff + j:coff + j1],
                            start=True, stop=True,
                        )
                    if route == 0:
                        nc.vector.tensor_reduce(
                            out=dist_sb[:, s:s + 1], in_=ps[:, :w],
                            axis=mybir.AxisListType.X, op=mybir.AluOpType.min,
                        )
                        if pending:
                            emit_ts(pending.pop(0))
                    else:
                        st = stp.tile([P, WSEG], F16, tag="st")
                        nc.scalar.activation(
                            out=st[:, :w], in_=ps[:, :w],
                            func=mybir.ActivationFunctionType.Relu,
                        )
                        if pending:
                            emit_ts(pending.pop(0))
                        pending.append((st[:, :w], w, s))
                    s += 1
                flush_pair(force=True)
                while pending:
                    emit_ts(pending.pop(0))

            # final relu (covers route-0 segments; min commutes with relu)
            nc.vector.tensor_scalar(
                out=dist_sb[:], in0=dist_sb[:], scalar1=0.0, scalar2=None,
                op0=mybir.AluOpType.max,
            )
            nc.sync.dma_start(out=dout[:], in_=dist_sb[:])

    nc.finalize()
    _cache[key] = nc
    return nc


# ------------------------------------------------------------------- driver

def _prepare(input1, input2):
    p1 = np.ascontiguousarray(np.asarray(input1, np.float32))
    p2 = np.ascontiguousarray(np.asarray(input2, np.float32))
    sides = []  # side 0: q=p1,c=p2 (dist1); side 1: q=p2,c=p1 (dist2)
    for (qa, ca) in ((p1, p2), (p2, p1)):
        binfo = [_bands(qa[b].astype(np.float64), ca[b].astype(np.float64))
                 for b in range(B)]
        plan = _plan_side(binfo)
        sides.append((binfo, plan, qa, ca))
    return sides


def run(input1, input2, trace=False):
    sides = _prepare(input1, input2)
    (binfo_a, plan_a, q_a, c_a) = sides[0]
    (binfo_b, plan_b, q_b, c_b) = sides[1]

    nc = build(plan_a["segs"], plan_b["segs"],
               plan_a["QTOT"], plan_b["QTOT"],
               plan_a["CTOT"], plan_b["CTOT"])
    global LAST_NC
    LAST_NC = nc

    in_maps = []
    for b in range(B):
        im = {}
        for name_q, name_c, (binfo, plan, qq, cc) in (
                ("qA", "cA", (binfo_a, plan_a, q_a, c_a)),
                ("qB", "cB", (binfo_b, plan_b, q_b, c_b))):
            qi, ci, blo, bhi = binfo[b]
            qs = qq[b][qi]
            cs = cc[b][ci]
            augq = _aug_q(qs)
            augc = _aug_c(cs)
            Q = np.empty((5, plan["QTOT"]), np.float32)
            Q[:, :N] = augq
            for g in range(plan["K"]):
                Q[:, N + g * P:N + (g + 1) * P] = augq[:, plan["lanes_out"][b, g]]
            C = np.empty((5, plan["CTOT"]), np.float32)
            off = 0
            for ch in range(NCH):
                W = int(plan["W_norm"][ch])
                l = int(plan["l_norm"][b, ch])
                C[:, off:off + W] = augc[:, l:l + W]
                off += W
            for g in range(plan["K"]):
                W = int(plan["W_out"][g])
                l = int(plan["l_out"][b, g])
                C[:, off:off + W] = augc[:, l:l + W]
                off += W
            im[name_q] = Q
            im[name_c] = C
        in_maps.append(im)

    res = run_bass_kernel_spmd(nc, in_maps, list(range(B)), trace=trace)

    # host merge
    sa = len(plan_a["segs"])
    dist1 = np.empty((B, N), np.float32)
    dist2 = np.empty((B, N), np.float32)
    for b in range(B):
        out = res.results[b]["dout"]  # [P, S]
        for side, (binfo, plan, dst) in enumerate(
                ((binfo_a, plan_a, dist1), (binfo_b, plan_b, dist2))):
            qi = binfo[b][0]
            segs = plan["segs"]
            cols = out[:, :sa] if side == 0 else out[:, sa:]
            acc = np.full(N, np.inf, np.float32)
            ids = np.empty((len(segs), P), np.int64)
            for s, (qoff, _, _) in enumerate(segs):
                if qoff < N:
                    ids[s] = qi[qoff:qoff + P]
                else:
                    g = (qoff - N) // P
                    ids[s] = qi[plan["lanes_out"][b, g]]
            np.minimum.at(acc, ids.reshape(-1), cols.T.reshape(-1))
            dst[b] = acc
    return (dist1, dist2), res


def kernel(input1, input2):
    (dist1, dist2), _ = run(input1, input2)
    return (dist1, dist2)


# revision 7
# speedup vs baseline: 1.0456x; 1.0446x over previous
"""Chamfer distance via banded exact nearest-neighbor for Trainium2.

Host side: for each batch and each direction (q=set1 vs c=set2 and the
swap), sort candidates by x and compute for every query a provable upper
bound u(q) on its NN distance (exact distance to the best of ~128 real
candidate points found via x-window + Morton-window).  The true NN then
provably lies in the sorted-candidate index band {m : |x_m - x_q| <= u(q)}.
The device only computes distance tiles over those bands.

Queries are processed in chunks of 128 (sorted order); a chunk's band is
the union of its members' bands, excluding "outlier" queries with wide
bands, which are re-processed in dedicated gathered outlier chunks.  Every
computed value is a distance to a real candidate, so a host-side min-merge
of all contributions per query is exact (up to fp32r matmul rounding).

Device per segment (band split into <=1024-wide pieces):
  - K=5 augmented matmul [5,128]x[5,W] -> PSUM fp32 (fp32r, 1 cycle/row)
  - route A (DVE): tensor_reduce min directly from PSUM -> dist col
  - route B (ACT+DVE): ACT Relu-cast PSUM->SBUF f16, then DVE
    tensor_scalar min-accum in 4x mode
  routes are balanced greedily between the ACT and DVE engines.
No staging of the full distance matrix, no transpose tail.
"""

import sys

sys.path.insert(0, "/opt/trn_rl_repo")

import numpy as np

import concourse.bass as bass  # noqa: F401
import concourse.mybir as mybir
import concourse.tile as tile
from concourse import bacc
from concourse.bass_utils import run_bass_kernel_spmd

B, N, M = 8, 8192, 8192
P = 128
NCH = N // P          # 64 normal chunks per side
GRAN = 128            # band width granularity (<=512 widths)
WSEG = 1024           # max segment width (psum tile = 2 banks)
T_OUT = 320           # per-query band width above which query is an outlier
KX = 192              # x-window candidates for u(q)
KM = 192              # morton-window candidates for u(q)

F32 = mybir.dt.float32
F32R = mybir.dt.float32r
F16 = mybir.dt.float16


# ----------------------------------------------------------------- host plan

def _morton(p):
    pm = (p - p.min(0)) / (p.max(0) - p.min(0) + 1e-12)
    g = np.minimum((pm * 1024).astype(np.int64), 1023)

    def spread(x):
        x = (x | (x << 16)) & 0x030000FF
        x = (x | (x << 8)) & 0x0300F00F
        x = (x | (x << 4)) & 0x030C30C3
        x = (x | (x << 2)) & 0x09249249
        return x

    return (spread(g[:, 0]) << 2) | (spread(g[:, 1]) << 1) | spread(g[:, 2])


def _bands(q, c):
    """q,c float64 [n,3]. Returns qi, ci, blo, bhi (bands of sorted queries
    in sorted-candidate index space, provably containing each query's NN)."""
    n, m = len(q), len(c)
    qi = np.argsort(q[:, 0], kind="stable")
    ci = np.argsort(c[:, 0], kind="stable")
    qs, cs = q[qi], c[ci]
    xq, xc = qs[:, 0], cs[:, 0]
    pos = np.searchsorted(xc, xq)
    lo = np.clip(pos - KX // 2, 0, m - KX)
    idx = lo[:, None] + np.arange(KX)[None, :]
    u2 = ((qs[:, None, :] - cs[idx]) ** 2).sum(-1).min(1)
    mq, mc = _morton(qs), _morton(cs)
    mci = np.argsort(mc)
    posm = np.searchsorted(mc[mci], mq)
    lom = np.clip(posm - KM // 2, 0, m - KM)
    idxm = mci[lom[:, None] + np.arange(KM)[None, :]]
    u2 = np.minimum(u2, ((qs[:, None, :] - cs[idxm]) ** 2).sum(-1).min(1))
    u = np.sqrt(u2) * (1 + 1e-9) + 1e-12
    blo = np.searchsorted(xc, xq - u, "left")
    bhi = np.searchsorted(xc, xq + u, "right")
    return qi, ci, blo, bhi


def _pad(w):
    w = int(w)
    if w <= 512:
        return max(256, -(-w // GRAN) * GRAN)
    return -(-w // 256) * 256


def _plan_side(batches):
    """batches: list of (qi, ci, blo, bhi) per batch.  Returns a shared chunk
    plan + per-batch gather/merge info.

    chunk list entries: (kind, index, W); per batch: per chunk candidate
    slice start l_b, and for outlier chunks the lane->sorted-query mapping.
    """
    nb = len(batches)
    # normal chunks
    W_norm = np.zeros(NCH, np.int64)
    l_norm = np.zeros((nb, NCH), np.int64)
    for b, (qi, ci, blo, bhi) in enumerate(batches):
        w = bhi - blo
        inl = w <= T_OUT
        for ch in range(NCH):
            s = slice(ch * P, (ch + 1) * P)
            msk = inl[s]
            if msk.any():
                l, h = blo[s][msk].min(), bhi[s][msk].max()
            else:
                l, h = blo[s].min(), blo[s].min() + GRAN
            l_norm[b, ch] = l
            W_norm[ch] = max(W_norm[ch], _pad(h - l))
    W_norm = np.minimum(W_norm, M)
    l_norm = np.minimum(l_norm, M - W_norm[None, :])
    np.clip(l_norm, 0, None, out=l_norm)

    # outlier groups per batch
    out_groups = []  # per batch: list of (sorted_positions[<=P], l, need)
    for b, (qi, ci, blo, bhi) in enumerate(batches):
        w = bhi - blo
        oq = np.where(w > T_OUT)[0]
        oq = oq[np.argsort(blo[oq], kind="stable")]
        groups = []
        for i in range(0, len(oq), P):
            grp = oq[i:i + P]
            l, h = blo[grp].min(), bhi[grp].max()
            groups.append((grp, int(l), int(h - l)))
        out_groups.append(groups)
    K = max(len(g) for g in out_groups)
    W_out = np.zeros(K, np.int64)
    l_out = np.zeros((nb, K), np.int64)
    lanes_out = np.zeros((nb, K, P), np.int64)  # sorted-query positions
    for b in range(nb):
        for g in range(K):
            if g < len(out_groups[b]):
                grp, l, need = out_groups[b][g]
            else:
                grp, l, need = np.array([0]), 0, GRAN
            lane = np.empty(P, np.int64)
            lane[:len(grp)] = grp
            lane[len(grp):] = grp[-1]
            lanes_out[b, g] = lane
            l_out[b, g] = l
            W_out[g] = max(W_out[g], _pad(need))
    W_out = np.minimum(W_out, M)
    l_out = np.minimum(l_out, M - W_out[None, :])
    np.clip(l_out, 0, None, out=l_out)

    # segment list (shared across batches)
    segs = []  # (qoff_rel, coff_rel, w)  qoff_rel in side Q tensor
    coff = 0
    for ch in range(NCH):
        Wc = int(W_norm[ch])
        for j in range(0, Wc, WSEG):
            segs.append((ch * P, coff + j, min(WSEG, Wc - j)))
        coff += Wc
    for g in range(K):
        Wc = int(W_out[g])
        for j in range(0, Wc, WSEG):
            segs.append((N + g * P, coff + j, min(WSEG, Wc - j)))
        coff += Wc

    return dict(K=K, QTOT=N + K * P, CTOT=coff, segs=segs,
                W_norm=W_norm, l_norm=l_norm,
                W_out=W_out, l_out=l_out, lanes_out=lanes_out)


def _aug_q(p):
    """p [n,3] fp32 -> [5,n]: x,y,z,|p|^2,1"""
    a = np.empty((5, len(p)), np.float32)
    a[0:3] = p.T
    a[3] = (p * p).sum(1)
    a[4] = 1.0
    return a


def _aug_c(p):
    """p [m,3] fp32 -> [5,m]: -2x,-2y,-2z,1,|p|^2"""
    a = np.empty((5, len(p)), np.float32)
    a[0:3] = -2.0 * p.T
    a[3] = 1.0
    a[4] = (p * p).sum(1)
    return a


def _gather_side(plan, qi, ci, q32, c32):
    """Build per-batch Q [5,QTOT] and C [5,CTOT] fp32 for one side/batch."""
    b = None  # noqa
    qs = q32[qi]
    cs = c32[ci]
    augq = _aug_q(qs)
    augc = _aug_c(cs)
    Q = np.empty((5, plan["QTOT"]), np.float32)
    Q[:, :N] = augq
    C = np.empty((5, plan["CTOT"]), np.float32)
    return Q, C, augq, augc, cs


# ------------------------------------------------------------- device build

_cache = {}


def build(segs_a, segs_b, qtot_a, qtot_b, ctot_a, ctot_b):
    key = (tuple(segs_a), tuple(segs_b), qtot_a, qtot_b, ctot_a, ctot_b)
    if key in _cache:
        return _cache[key]
    CMAX = max(ctot_a, ctot_b)
    S = len(segs_a) + len(segs_b)

    act_t = 0.0
    dve_t = 0.0
    routes = []
    for (_, _, w) in list(segs_a) + list(segs_b):
        c_dir = w * 1.042 + 190.0
        c_act = w * 0.99 + (160.0 if w <= 512 else 245.0)
        c_tsd = w * 0.26 + 130.0
        m_dir = max(act_t, dve_t + c_dir)
        m_act = max(act_t + c_act, dve_t + c_tsd)
        if m_act <= m_dir:
            routes.append(1)
            act_t += c_act
            dve_t += c_tsd
        else:
            routes.append(0)
            dve_t += c_dir

    nc = bacc.Bacc()
    qA = nc.declare_dram_parameter("qA", [5, qtot_a], F32R, isOutput=False)
    cA = nc.declare_dram_parameter("cA", [5, ctot_a], F32R, isOutput=False)
    qB = nc.declare_dram_parameter("qB", [5, qtot_b], F32R, isOutput=False)
    cB = nc.declare_dram_parameter("cB", [5, ctot_b], F32R, isOutput=False)
    dout = nc.declare_dram_parameter("dout", [P, S], F32, isOutput=True)

    with tile.TileContext(nc) as tc:
        with tc.tile_pool(name="const", bufs=1) as const, \
             tc.tile_pool(name="st", bufs=4) as stp, \
             tc.tile_pool(name="psum1", bufs=2, space="PSUM") as psum1, \
             tc.tile_pool(name="psum2", bufs=3, space="PSUM") as psum2:

            QMAX = max(qtot_a, qtot_b)
            q_sb = const.tile([5, QMAX], F32R)
            c_sb = const.tile([5, CMAX], F32R)
            dist_sb = const.tile([P, S], F32)

            s = 0
            qi_ = 0
            for phase, (segs, qT, qtot, cT, ctot) in enumerate(
                    ((segs_a, qA, qtot_a, cA, ctot_a),
                     (segs_b, qB, qtot_b, cB, ctot_b))):
                nc.sync.dma_start(out=q_sb[:, :qtot], in_=qT[:])
                # candidate DMA in graded pieces so early chunks start sooner
                cuts = [0, ctot // 16, ctot // 8, ctot // 4, ctot // 2, ctot]
                for a, b2 in zip(cuts[:-1], cuts[1:]):
                    if a < b2:
                        nc.sync.dma_start(out=c_sb[:, a:b2], in_=cT[:, a:b2])

                def emit_ts(p):
                    ap_, w_, s_ = p
                    nc.vector.tensor_scalar(
                        out=ap_, in0=ap_,
                        scalar1=0.0, scalar2=None,
                        op0=mybir.AluOpType.max, op1=mybir.AluOpType.min,
                        accum_out=dist_sb[:, s_:s_ + 1],
                    )

                pending = []   # delayed route-1 TS ops (DVE keeps busy
                # while ACT casts); route-1 w<=512 segs are PAIRED into one
                # 2-bank psum tile with a single ACT cast (the slot gap casts
                # stale psum data that no TS ever reads)
                pbuf = []

                def flush_pair(force=False):
                    if not pbuf or (len(pbuf) < 2 and not force):
                        return
                    grp = pbuf[:2]
                    del pbuf[:len(grp)]
                    ps = psum2.tile([P, WSEG], F32, tag="ps2")
                    for k, (qoff_, coff_, w_, s_) in enumerate(grp):
                        nc.tensor.matmul(
                            out=ps[:, k * 512:k * 512 + w_],
                            lhsT=q_sb[:, qoff_:qoff_ + P],
                            rhs=c_sb[:, coff_:coff_ + w_],
                            start=True, stop=True,
                        )
                    span = (len(grp) - 1) * 512 + grp[-1][2]
                    st = stp.tile([P, WSEG], F16, tag="st")
                    nc.scalar.activation(
                        out=st[:, :span], in_=ps[:, :span],
                        func=mybir.ActivationFunctionType.Relu,
                    )
                    while pending:
                        emit_ts(pending.pop(0))
                    for k, (qoff_, coff_, w_, s_) in enumerate(grp):
                        pending.append((st[:, k * 512:k * 512 + w_], w_, s_))

                for (qoff, coff, w) in segs:
                    route = routes[s]
                    if route == 1 and w <= 512:
                        pbuf.append((qoff, coff, w, s))
                        flush_pair()
                        s += 1
                        continue
                    if w <= 512:
                        ps = psum1.tile([P, 512], F32, tag="ps1")
                    else:
                        ps = psum2.tile([P, WSEG], F32, tag="ps2")
                    for j in range(0, w, 512):
                        j1 = min(j + 512, w)
                        nc.tensor.matmul(
                            out=ps[:, j:j1],
                            lhsT=q_sb[:, qoff:qoff + P],
                            rhs=c_sb[:, coff + j:coff + j1],
                            start=True, stop=True,
                        )
                    if route == 0:
                        nc.vector.tensor_reduce(
                            out=dist_sb[:, s:s + 1], in_=ps[:, :w],
                            axis=mybir.AxisListType.X, op=mybir.AluOpType.min,
                        )
                        if pending:
                            emit_ts(pending.pop(0))
                    else:
                        st = stp.tile([P, WSEG], F16, tag="st")
                        nc.scalar.activation(
                            out=st[:, :w], in_=ps[:, :w],
                            func=mybir.ActivationFunctionType.Relu,
                        )
                        if pending:
                            emit_ts(pending.pop(0))
                        pending.append((st[:, :w], w, s))
                    s += 1
                flush_pair(force=True)
                while pending:
                    emit_ts(pending.pop(0))

            # final relu (covers route-0 segments; min commutes with relu)
            nc.vector.tensor_scalar(
                out=dist_sb[:], in0=dist_sb[:], scalar1=0.0, scalar2=None,
                op0=mybir.AluOpType.max,
            )
            nc.sync.dma_start(out=dout[:], in_=dist_sb[:])

    nc.finalize()
    _cache[key] = nc
    return nc


# ------------------------------------------------------------------- driver

def _prepare(input1, input2):
    p1 = np.ascontiguousarray(np.asarray(input1, np.float32))
    p2 = np.ascontiguousarray(np.asarray(input2, np.float32))
    sides = []  # side 0: q=p1,c=p2 (dist1); side 1: q=p2,c=p1 (dist2)
    for (qa, ca) in ((p1, p2), (p2, p1)):
        binfo = [_bands(qa[b].astype(np.float64), ca[b].astype(np.float64))
                 for b in range(B)]
        plan = _plan_side(binfo)
        sides.append((binfo, plan, qa, ca))
    return sides


def run(input1, input2, trace=False):
    sides = _prepare(input1, input2)
    (binfo_a, plan_a, q_a, c_a) = sides[0]
    (binfo_b, plan_b, q_b, c_b) = sides[1]

    nc = build(plan_a["segs"], plan_b["segs"],
               plan_a["QTOT"], plan_b["QTOT"],
               plan_a["CTOT"], plan_b["CTOT"])
    global LAST_NC
    LAST_NC = nc

    in_maps = []
    for b in range(B):
        im = {}
        for name_q, name_c, (binfo, plan, qq, cc) in (
                ("qA", "cA", (binfo_a, plan_a, q_a, c_a)),
                ("qB", "cB", (binfo_b, plan_b, q_b, c_b))):
            qi, ci, blo, bhi = binfo[b]
            qs = qq[b][qi]
            cs = cc[b][ci]
            augq = _aug_q(qs)
            augc = _aug_c(cs)
            Q = np.empty((5, plan["QTOT"]), np.float32)
            Q[:, :N] = augq
            for g in range(plan["K"]):
                Q[:, N + g * P:N + (g + 1) * P] = augq[:, plan["lanes_out"][b, g]]
            C = np.empty((5, plan["CTOT"]), np.float32)
            off = 0
            for ch in range(NCH):
                W = int(plan["W_norm"][ch])
                l = int(plan["l_norm"][b, ch])
                C[:, off:off + W] = augc[:, l:l + W]
                off += W
            for g in range(plan["K"]):
                W = int(plan["W_out"][g])
                l = int(plan["l_out"][b, g])
                C[:, off:off + W] = augc[:, l:l + W]
                off += W
            im[name_q] = Q
            im[name_c] = C
        in_maps.append(im)

    res = run_bass_kernel_spmd(nc, in_maps, list(range(B)), trace=trace)

    # host merge
    sa = len(plan_a["segs"])
    dist1 = np.empty((B, N), np.float32)
    dist2 = np.empty((B, N), np.float32)
    for b in range(B):
        out = res.results[b]["dout"]  # [P, S]
        for side, (binfo, plan, dst) in enumerate(
                ((binfo_a, plan_a, dist1), (binfo_b, plan_b, dist2))):
            qi = binfo[b][0]
            segs = plan["segs"]
            cols = out[:, :sa] if side == 0 else out[:, sa:]
            acc = np.full(N, np.inf, np.float32)
            ids = np.empty((len(segs), P), np.int64)
            for s, (qoff, _, _) in enumerate(segs):
                if qoff < N:
                    ids[s] = qi[qoff:qoff + P]
                else:
                    g = (qoff - N) // P
                    ids[s] = qi[plan["lanes_out"][b, g]]
            np.minimum.at(acc, ids.reshape(-1), cols.T.reshape(-1))
            dst[b] = acc
    return (dist1, dist2), res


def kernel(input1, input2):
    (dist1, dist2), _ = run(input1, input2)
    return (dist1, dist2)


# revision 8
# speedup vs baseline: 1.0703x; 1.0236x over previous
"""Chamfer distance via banded exact nearest-neighbor for Trainium2.

Host side: for each batch and each direction (q=set1 vs c=set2 and the
swap), sort candidates by x and compute for every query a provable upper
bound u(q) on its NN distance (exact distance to the best of ~128 real
candidate points found via x-window + Morton-window).  The true NN then
provably lies in the sorted-candidate index band {m : |x_m - x_q| <= u(q)}.
The device only computes distance tiles over those bands.

Queries are processed in chunks of 128 (sorted order); a chunk's band is
the union of its members' bands, excluding "outlier" queries with wide
bands, which are re-processed in dedicated gathered outlier chunks.  Every
computed value is a distance to a real candidate, so a host-side min-merge
of all contributions per query is exact (up to fp32r matmul rounding).

Device per segment (band split into <=1024-wide pieces):
  - K=5 augmented matmul [5,128]x[5,W] -> PSUM fp32 (fp32r, 1 cycle/row)
  - route A (DVE): tensor_reduce min directly from PSUM -> dist col
  - route B (ACT+DVE): ACT Relu-cast PSUM->SBUF f16, then DVE
    tensor_scalar min-accum in 4x mode
  routes are balanced greedily between the ACT and DVE engines.
No staging of the full distance matrix, no transpose tail.
"""

import sys

sys.path.insert(0, "/opt/trn_rl_repo")

import numpy as np

import concourse.bass as bass  # noqa: F401
import concourse.mybir as mybir
import concourse.tile as tile
from concourse import bacc
from concourse.bass_utils import run_bass_kernel_spmd

B, N, M = 8, 8192, 8192
P = 128
NCH = N // P          # 64 normal chunks per side
GRAN = 128            # band width granularity (<=512 widths)
WSEG = 1024           # max segment width (psum tile = 2 banks)
T_OUT = 320           # per-query band width above which query is an outlier
KX = 192              # x-window candidates for u(q)
KM = 192              # morton-window candidates for u(q)

F32 = mybir.dt.float32
F32R = mybir.dt.float32r
F16 = mybir.dt.float16


# ----------------------------------------------------------------- host plan

def _morton(p):
    pm = (p - p.min(0)) / (p.max(0) - p.min(0) + 1e-12)
    g = np.minimum((pm * 1024).astype(np.int64), 1023)

    def spread(x):
        x = (x | (x << 16)) & 0x030000FF
        x = (x | (x << 8)) & 0x0300F00F
        x = (x | (x << 4)) & 0x030C30C3
        x = (x | (x << 2)) & 0x09249249
        return x

    return (spread(g[:, 0]) << 2) | (spread(g[:, 1]) << 1) | spread(g[:, 2])


def _bands(q, c):
    """q,c float64 [n,3]. Returns qi, ci, blo, bhi (bands of sorted queries
    in sorted-candidate index space, provably containing each query's NN)."""
    n, m = len(q), len(c)
    qi = np.argsort(q[:, 0], kind="stable")
    ci = np.argsort(c[:, 0], kind="stable")
    qs, cs = q[qi], c[ci]
    xq, xc = qs[:, 0], cs[:, 0]
    pos = np.searchsorted(xc, xq)
    lo = np.clip(pos - KX // 2, 0, m - KX)
    idx = lo[:, None] + np.arange(KX)[None, :]
    u2 = ((qs[:, None, :] - cs[idx]) ** 2).sum(-1).min(1)
    mq, mc = _morton(qs), _morton(cs)
    mci = np.argsort(mc)
    posm = np.searchsorted(mc[mci], mq)
    lom = np.clip(posm - KM // 2, 0, m - KM)
    idxm = mci[lom[:, None] + np.arange(KM)[None, :]]
    u2 = np.minimum(u2, ((qs[:, None, :] - cs[idxm]) ** 2).sum(-1).min(1))
    u = np.sqrt(u2) * (1 + 1e-9) + 1e-12
    blo = np.searchsorted(xc, xq - u, "left")
    bhi = np.searchsorted(xc, xq + u, "right")
    return qi, ci, blo, bhi


def _pad(w):
    w = int(w)
    if w <= 512:
        return max(256, -(-w // GRAN) * GRAN)
    return -(-w // 256) * 256


def _plan_side(batches):
    """batches: list of (qi, ci, blo, bhi) per batch.  Returns a shared chunk
    plan + per-batch gather/merge info.

    chunk list entries: (kind, index, W); per batch: per chunk candidate
    slice start l_b, and for outlier chunks the lane->sorted-query mapping.
    """
    nb = len(batches)
    # normal chunks
    W_norm = np.zeros(NCH, np.int64)
    l_norm = np.zeros((nb, NCH), np.int64)
    for b, (qi, ci, blo, bhi) in enumerate(batches):
        w = bhi - blo
        inl = w <= T_OUT
        for ch in range(NCH):
            s = slice(ch * P, (ch + 1) * P)
            msk = inl[s]
            if msk.any():
                l, h = blo[s][msk].min(), bhi[s][msk].max()
            else:
                l, h = blo[s].min(), blo[s].min() + GRAN
            l_norm[b, ch] = l
            W_norm[ch] = max(W_norm[ch], _pad(h - l))
    W_norm = np.minimum(W_norm, M)
    l_norm = np.minimum(l_norm, M - W_norm[None, :])
    np.clip(l_norm, 0, None, out=l_norm)

    # outlier groups per batch
    out_groups = []  # per batch: list of (sorted_positions[<=P], l, need)
    for b, (qi, ci, blo, bhi) in enumerate(batches):
        w = bhi - blo
        oq = np.where(w > T_OUT)[0]
        oq = oq[np.argsort(blo[oq], kind="stable")]
        groups = []
        for i in range(0, len(oq), P):
            grp = oq[i:i + P]
            l, h = blo[grp].min(), bhi[grp].max()
            groups.append((grp, int(l), int(h - l)))
        out_groups.append(groups)
    K = max(len(g) for g in out_groups)
    W_out = np.zeros(K, np.int64)
    l_out = np.zeros((nb, K), np.int64)
    lanes_out = np.zeros((nb, K, P), np.int64)  # sorted-query positions
    for b in range(nb):
        for g in range(K):
            if g < len(out_groups[b]):
                grp, l, need = out_groups[b][g]
            else:
                grp, l, need = np.array([0]), 0, GRAN
            lane = np.empty(P, np.int64)
            lane[:len(grp)] = grp
            lane[len(grp):] = grp[-1]
            lanes_out[b, g] = lane
            l_out[b, g] = l
            W_out[g] = max(W_out[g], _pad(need))
    W_out = np.minimum(W_out, M)
    l_out = np.minimum(l_out, M - W_out[None, :])
    np.clip(l_out, 0, None, out=l_out)

    # segment list (shared across batches)
    segs = []  # (qoff_rel, coff_rel, w)  qoff_rel in side Q tensor
    coff = 0
    for ch in range(NCH):
        Wc = int(W_norm[ch])
        for j in range(0, Wc, WSEG):
            segs.append((ch * P, coff + j, min(WSEG, Wc - j)))
        coff += Wc
    for g in range(K):
        Wc = int(W_out[g])
        for j in range(0, Wc, WSEG):
            segs.append((N + g * P, coff + j, min(WSEG, Wc - j)))
        coff += Wc

    return dict(K=K, QTOT=N + K * P, CTOT=coff, segs=segs,
                W_norm=W_norm, l_norm=l_norm,
                W_out=W_out, l_out=l_out, lanes_out=lanes_out)


def _aug_q(p):
    """p [n,3] fp32 -> [5,n]: x,y,z,|p|^2,1"""
    a = np.empty((5, len(p)), np.float32)
    a[0:3] = p.T
    a[3] = (p * p).sum(1)
    a[4] = 1.0
    return a


def _aug_c(p):
    """p [m,3] fp32 -> [5,m]: -2x,-2y,-2z,1,|p|^2"""
    a = np.empty((5, len(p)), np.float32)
    a[0:3] = -2.0 * p.T
    a[3] = 1.0
    a[4] = (p * p).sum(1)
    return a


def _gather_side(plan, qi, ci, q32, c32):
    """Build per-batch Q [5,QTOT] and C [5,CTOT] fp32 for one side/batch."""
    b = None  # noqa
    qs = q32[qi]
    cs = c32[ci]
    augq = _aug_q(qs)
    augc = _aug_c(cs)
    Q = np.empty((5, plan["QTOT"]), np.float32)
    Q[:, :N] = augq
    C = np.empty((5, plan["CTOT"]), np.float32)
    return Q, C, augq, augc, cs


# ------------------------------------------------------------- device build

_cache = {}


def build(segs_a, segs_b, qtot_a, qtot_b, ctot_a, ctot_b):
    key = (tuple(segs_a), tuple(segs_b), qtot_a, qtot_b, ctot_a, ctot_b)
    if key in _cache:
        return _cache[key]
    CMAX = max(ctot_a, ctot_b)
    S = len(segs_a) + len(segs_b)

    act_t = 0.0
    dve_t = 0.0
    routes = []
    for (_, _, w) in list(segs_a) + list(segs_b):
        c_dir = w * 1.042 + 190.0
        c_act = w * 1.12 + (160.0 if w <= 512 else 245.0)
        c_tsd = w * 0.26 + 130.0
        m_dir = max(act_t, dve_t + c_dir)
        m_act = max(act_t + c_act, dve_t + c_tsd)
        if m_act <= m_dir:
            routes.append(1)
            act_t += c_act
            dve_t += c_tsd
        else:
            routes.append(0)
            dve_t += c_dir

    nc = bacc.Bacc()
    qA = nc.declare_dram_parameter("qA", [5, qtot_a], F32R, isOutput=False)
    cA = nc.declare_dram_parameter("cA", [5, ctot_a], F32R, isOutput=False)
    qB = nc.declare_dram_parameter("qB", [5, qtot_b], F32R, isOutput=False)
    cB = nc.declare_dram_parameter("cB", [5, ctot_b], F32R, isOutput=False)
    dout = nc.declare_dram_parameter("dout", [P, S], F32, isOutput=True)

    with tile.TileContext(nc) as tc:
        with tc.tile_pool(name="const", bufs=1) as const, \
             tc.tile_pool(name="st", bufs=4) as stp, \
             tc.tile_pool(name="psum1", bufs=2, space="PSUM") as psum1, \
             tc.tile_pool(name="psum2", bufs=3, space="PSUM") as psum2:

            QMAX = max(qtot_a, qtot_b)
            q_sb = const.tile([5, QMAX], F32R)
            c_sb = const.tile([5, CMAX], F32R)
            dist_sb = const.tile([P, S], F32)

            s = 0
            qi_ = 0
            for phase, (segs, qT, qtot, cT, ctot) in enumerate(
                    ((segs_a, qA, qtot_a, cA, ctot_a),
                     (segs_b, qB, qtot_b, cB, ctot_b))):
                nc.sync.dma_start(out=q_sb[:, :qtot], in_=qT[:])
                # candidate DMA in graded pieces so early chunks start sooner
                cuts = [0, ctot // 16, ctot // 8, ctot // 4, ctot // 2, ctot]
                for a, b2 in zip(cuts[:-1], cuts[1:]):
                    if a < b2:
                        nc.sync.dma_start(out=c_sb[:, a:b2], in_=cT[:, a:b2])

                def emit_ts(p):
                    ap_, w_, s_ = p
                    nc.vector.tensor_scalar(
                        out=ap_, in0=ap_,
                        scalar1=0.0, scalar2=None,
                        op0=mybir.AluOpType.max, op1=mybir.AluOpType.min,
                        accum_out=dist_sb[:, s_:s_ + 1],
                    )

                pending = []   # delayed route-1 TS ops (DVE keeps busy
                # while ACT casts); route-1 w<=512 segs are PAIRED into one
                # 2-bank psum tile with a single ACT cast (the slot gap casts
                # stale psum data that no TS ever reads)
                pbuf = []

                def flush_pair(force=False):
                    if not pbuf or (len(pbuf) < 2 and not force):
                        return
                    grp = pbuf[:2]
                    del pbuf[:len(grp)]
                    ps = psum2.tile([P, WSEG], F32, tag="ps2")
                    for k, (qoff_, coff_, w_, s_) in enumerate(grp):
                        nc.tensor.matmul(
                            out=ps[:, k * 512:k * 512 + w_],
                            lhsT=q_sb[:, qoff_:qoff_ + P],
                            rhs=c_sb[:, coff_:coff_ + w_],
                            start=True, stop=True,
                        )
                    span = (len(grp) - 1) * 512 + grp[-1][2]
                    st = stp.tile([P, WSEG], F16, tag="st")
                    nc.scalar.activation(
                        out=st[:, :span], in_=ps[:, :span],
                        func=mybir.ActivationFunctionType.Relu,
                    )
                    while pending:
                        emit_ts(pending.pop(0))
                    for k, (qoff_, coff_, w_, s_) in enumerate(grp):
                        pending.append((st[:, k * 512:k * 512 + w_], w_, s_))

                for (qoff, coff, w) in segs:
                    route = routes[s]
                    if route == 1 and w <= 512:
                        pbuf.append((qoff, coff, w, s))
                        flush_pair()
                        s += 1
                        continue
                    if w <= 512:
                        ps = psum1.tile([P, 512], F32, tag="ps1")
                    else:
                        ps = psum2.tile([P, WSEG], F32, tag="ps2")
                    for j in range(0, w, 512):
                        j1 = min(j + 512, w)
                        nc.tensor.matmul(
                            out=ps[:, j:j1],
                            lhsT=q_sb[:, qoff:qoff + P],
                            rhs=c_sb[:, coff + j:coff + j1],
                            start=True, stop=True,
                        )
                    if route == 0:
                        nc.vector.tensor_reduce(
                            out=dist_sb[:, s:s + 1], in_=ps[:, :w],
                            axis=mybir.AxisListType.X, op=mybir.AluOpType.min,
                        )
                        if pending:
                            emit_ts(pending.pop(0))
                    else:
                        st = stp.tile([P, WSEG], F16, tag="st")
                        nc.scalar.activation(
                            out=st[:, :w], in_=ps[:, :w],
                            func=mybir.ActivationFunctionType.Relu,
                        )
                        if pending:
                            emit_ts(pending.pop(0))
                        pending.append((st[:, :w], w, s))
                    s += 1
                flush_pair(force=True)
                while pending:
                    emit_ts(pending.pop(0))

            # final relu (covers route-0 segments; min commutes with relu)
            nc.vector.tensor_scalar(
                out=dist_sb[:], in0=dist_sb[:], scalar1=0.0, scalar2=None,
                op0=mybir.AluOpType.max,
            )
            nc.sync.dma_start(out=dout[:], in_=dist_sb[:])

    nc.finalize()
    _cache[key] = nc
    return nc


# ------------------------------------------------------------------- driver

def _prepare(input1, input2):
    p1 = np.ascontiguousarray(np.asarray(input1, np.float32))
    p2 = np.ascontiguousarray(np.asarray(input2, np.float32))
    sides = []  # side 0: q=p1,c=p2 (dist1); side 1: q=p2,c=p1 (dist2)
    for (qa, ca) in ((p1, p2), (p2, p1)):
        binfo = [_bands(qa[b].astype(np.float64), ca[b].astype(np.float64))
                 for b in range(B)]
        plan = _plan_side(binfo)
        sides.append((binfo, plan, qa, ca))
    return sides


def run(input1, input2, trace=False):
    sides = _prepare(input1, input2)
    (binfo_a, plan_a, q_a, c_a) = sides[0]
    (binfo_b, plan_b, q_b, c_b) = sides[1]

    nc = build(plan_a["segs"], plan_b["segs"],
               plan_a["QTOT"], plan_b["QTOT"],
               plan_a["CTOT"], plan_b["CTOT"])
    global LAST_NC
    LAST_NC = nc

    in_maps = []
    for b in range(B):
        im = {}
        for name_q, name_c, (binfo, plan, qq, cc) in (
                ("qA", "cA", (binfo_a, plan_a, q_a, c_a)),
                ("qB", "cB", (binfo_b, plan_b, q_b, c_b))):
            qi, ci, blo, bhi = binfo[b]
            qs = qq[b][qi]
            cs = cc[b][ci]
            augq = _aug_q(qs)
            augc = _aug_c(cs)
            Q = np.empty((5, plan["QTOT"]), np.float32)
            Q[:, :N] = augq
            for g in range(plan["K"]):
                Q[:, N + g * P:N + (g + 1) * P] = augq[:, plan["lanes_out"][b, g]]
            C = np.empty((5, plan["CTOT"]), np.float32)
            off = 0
            for ch in range(NCH):
                W = int(plan["W_norm"][ch])
                l = int(plan["l_norm"][b, ch])
                C[:, off:off + W] = augc[:, l:l + W]
                off += W
            for g in range(plan["K"]):
                W = int(plan["W_out"][g])
                l = int(plan["l_out"][b, g])
                C[:, off:off + W] = augc[:, l:l + W]
                off += W
            im[name_q] = Q
            im[name_c] = C
        in_maps.append(im)

    res = run_bass_kernel_spmd(nc, in_maps, list(range(B)), trace=trace)

    # host merge
    sa = len(plan_a["segs"])
    dist1 = np.empty((B, N), np.float32)
    dist2 = np.empty((B, N), np.float32)
    for b in range(B):
        out = res.results[b]["dout"]  # [P, S]
        for side, (binfo, plan, dst) in enumerate(
                ((binfo_a, plan_a, dist1), (binfo_b, plan_b, dist2))):
            qi = binfo[b][0]
            segs = plan["segs"]
            cols = out[:, :sa] if side == 0 else out[:, sa:]
            acc = np.full(N, np.inf, np.float32)
            ids = np.empty((len(segs), P), np.int64)
            for s, (qoff, _, _) in enumerate(segs):
                if qoff < N:
                    ids[s] = qi[qoff:qoff + P]
                else:
                    g = (qoff - N) // P
                    ids[s] = qi[plan["lanes_out"][b, g]]
            np.minimum.at(acc, ids.reshape(-1), cols.T.reshape(-1))
            dst[b] = acc
    return (dist1, dist2), res


def kernel(input1, input2):
    (dist1, dist2), _ = run(input1, input2)
    return (dist1, dist2)
